# revision 1
# baseline (speedup 1.0000x reference)
"""ConvCapsuleLayer Trainium2 kernel: 5x5 SAME conv + 3-iter dynamic routing.

Sharding: 8 cores = batch(4) x H-halves(2). Per core: 5 images (4 input
capsules + their sum) of [68 rows, 32 atoms, 132 cols] (fp32r, host
pre-rounded), conv via quad-stacked K=128 matmuls, routing on DVE/GPSIMD/ACT
with positions on partitions.
"""
import numpy as np
from contextlib import ExitStack

import concourse.bass as bass
import concourse.tile as tile
from concourse import bacc, mybir
from concourse.bass_utils import run_bass_kernel_spmd

KK = 5
CI, CO, A = 4, 8, 32
COA = CO * A  # 256
NROWS = 64  # output rows per core
BLOCK = 6
F32 = mybir.dt.float32
F32R = mybir.dt.float32r
MULT = mybir.AluOpType.mult
ADD = mybir.AluOpType.add
AX = mybir.AxisListType.X
AF = mybir.ActivationFunctionType


def round_fp32r(x):
    """Round-to-nearest-even at 11 explicit mantissa bits (matches TRN2 fp32r)."""
    b = np.ascontiguousarray(x, np.float32).view(np.uint32)
    mask = np.uint32((1 << 12) - 1)
    half = np.uint32(1 << 11)
    frac = b & mask
    b2 = b & ~mask
    add = np.where(
        (frac > half) | ((frac == half) & (((b2 >> np.uint32(12)) & 1) == 1)),
        np.uint32(1 << 12),
        np.uint32(0),
    ).astype(np.uint32)
    return (b2 + add).view(np.float32)


def _blocks():
    out = []
    r0 = 0
    while r0 < NROWS:
        out.append((r0, min(BLOCK, NROWS - r0)))
        r0 += BLOCK
    return out


def build_program():
    nc = bacc.Bacc("TRN2", target_bir_lowering=False, debug=False, num_devices=1)

    xs_d = nc.dram_tensor("xs", [5, 68, 32, 132], F32R, kind="ExternalInput").ap()
    wstk_d = nc.dram_tensor("wstk", [128, 40 * COA], F32R, kind="ExternalInput").ap()
    bt_d = nc.dram_tensor("bt", [128, COA], F32, kind="ExternalInput").ap()
    out_d = nc.dram_tensor("out", [NROWS, 128, COA], F32, kind="ExternalOutput").ap()

    with tile.TileContext(nc) as tc, ExitStack() as ctx:
        cpool = ctx.enter_context(tc.tile_pool(name="const", bufs=1))
        vpool = ctx.enter_context(tc.tile_pool(name="vrows", bufs=8))
        tpool = ctx.enter_context(tc.tile_pool(name="tmps", bufs=2))
        spool = ctx.enter_context(tc.tile_pool(name="smalls", bufs=3))
        opool = ctx.enter_context(tc.tile_pool(name="outs", bufs=3))
        pspool = ctx.enter_context(tc.tile_pool(name="ps", bufs=8, space="PSUM"))

        # constants / inputs resident in SBUF
        quads = cpool.tile([128, 85 * 132], F32R)  # quad (i,q) at col (i*17+q)*132
        for i in range(5):
            for q in range(17):
                src = xs_d[i, 4 * q : 4 * q + 4].rearrange("r a w -> (r a) w")
                nc.sync.dma_start(quads[:, (i * 17 + q) * 132 : (i * 17 + q + 1) * 132], src)
        wstk = cpool.tile([128, 40 * COA], F32R)
        nc.sync.dma_start(wstk[:], wstk_d[:])
        btile = cpool.tile([128, COA], F32)
        nc.sync.dma_start(btile[:], bt_d[:])

        def wslice(m, sgn, kw):
            o = ((m * 2 + sgn) * KK + kw) * COA
            return wstk[:, o : o + COA]

        def qwin(i, q, kw):
            o = (i * 17 + q) * 132 + kw
            return quads[:, o : o + 128]

        def routing(r, vrow):
            """Routing for one output row; vrow [128, 5*COA] (ci-major + S)."""
            V = vrow[:, 0 : CI * COA]
            S = vrow[:, CI * COA : 5 * COA]
            v_cca = V.rearrange("p (ci co a) -> p ci co a", ci=CI, co=CO)
            v_cac = V.rearrange("p (ci co a) -> p co a ci", ci=CI, co=CO)

            def squash_scale(preact, tag):
                # returns s [128, CO]: n/(1+n^2), n^2 = sum_a preact^2
                sq = spool.tile([128, COA], F32, tag="sq")
                nc.scalar.activation(sq[:], preact, AF.Square)
                n2 = spool.tile([128, CO], F32, tag="n2")
                nc.vector.tensor_reduce(
                    n2[:].rearrange("p c -> p c"),
                    sq[:].rearrange("p (co a) -> p co a", co=CO),
                    AX, ADD,
                )
                rtn = spool.tile([128, CO], F32, tag="rtn")
                nc.scalar.activation(rtn[:], n2[:], AF.Sqrt)
                np1 = spool.tile([128, CO], F32, tag="np1")
                nc.scalar.add(np1[:], n2[:], 1.0)
                dinv = spool.tile([128, CO], F32, tag="dinv")
                nc.vector.reciprocal(dinv[:], np1[:])
                s = spool.tile([128, CO], F32, tag="s" + tag)
                nc.vector.tensor_tensor(s[:], rtn[:], dinv[:], MULT)
                return s

            def agreement(preact, s, tag):
                # returns contribution s[co] * sum_a V*preact : [128, CI*CO]
                t = tpool.tile([128, CI * COA], F32, tag="t")
                pb = preact.rearrange("p (co a) -> p co a", co=CO).broadcast_to(
                    (128, CO, A, CI)
                )
                nc.vector.tensor_tensor(
                    t[:].rearrange("p (ci co a) -> p co a ci", ci=CI, co=CO),
                    v_cac, pb, MULT,
                )
                lraw = spool.tile([128, CI * CO], F32, tag="lraw")
                nc.vector.tensor_reduce(
                    lraw[:].rearrange("p c -> p c"),
                    t[:].rearrange("p (ci co a) -> p ci co a", ci=CI, co=CO),
                    AX, ADD,
                )
                lc = spool.tile([128, CI * CO], F32, tag="lc" + tag)
                sb = s[:].rearrange("p co -> p co").broadcast_to((128, CO, CI))
                nc.vector.tensor_tensor(
                    lc[:].rearrange("p (ci co) -> p co ci", ci=CI),
                    lraw[:].rearrange("p (ci co) -> p co ci", ci=CI),
                    sb, MULT,
                )
                return lc

            def softmax_route(l, tag):
                # route = exp(l)/sum_co exp(l) (no max-sub; logits are moderate)
                e = spool.tile([128, CI * CO], F32, tag="e")
                nc.scalar.activation(e[:], l[:], AF.Exp)
                Z = spool.tile([128, CI], F32, tag="Z")
                nc.vector.tensor_reduce(
                    Z[:].rearrange("p c -> p c"),
                    e[:].rearrange("p (ci co) -> p ci co", ci=CI),
                    AX, ADD,
                )
                zi = spool.tile([128, CI], F32, tag="zi")
                nc.vector.reciprocal(zi[:], Z[:])
                rt = spool.tile([128, CI * CO], F32, tag="rt" + tag)
                zb = zi[:].rearrange("p ci -> p ci").broadcast_to((128, CI, CO))
                nc.vector.tensor_tensor(
                    rt[:].rearrange("p (ci co) -> p ci co", ci=CI),
                    e[:].rearrange("p (ci co) -> p ci co", ci=CI),
                    zb, MULT,
                )
                return rt

            def weighted_preact(rt, tag):
                # preact = sum_ci rt[ci,co]*V[ci,co,a] + b  (mul on GP, tree on GP+DVE)
                rV = tpool.tile([128, CI * COA], F32, tag="rV")
                rb = rt[:].rearrange("p (ci co) -> p ci co", ci=CI).broadcast_to(
                    (128, CI, CO, A)
                )
                nc.gpsimd.tensor_tensor(
                    rV[:].rearrange("p (ci co a) -> p ci co a", ci=CI, co=CO),
                    v_cca, rb, MULT,
                )
                h1 = spool.tile([128, COA], F32, tag="h1")
                h2 = spool.tile([128, COA], F32, tag="h2")
                nc.vector.tensor_add(h1[:], rV[:, 0:COA], rV[:, COA : 2 * COA])
                nc.vector.tensor_add(h2[:], rV[:, 2 * COA : 3 * COA], rV[:, 3 * COA : 4 * COA])
                h12 = spool.tile([128, COA], F32, tag="h12")
                nc.vector.tensor_add(h12[:], h1[:], h2[:])
                pre = spool.tile([128, COA], F32, tag="pre" + tag)
                nc.vector.tensor_tensor(pre[:], h12[:], btile[:], ADD)
                return pre

            # iter 1: uniform route -> preact1 = S/8 + b
            pre1 = spool.tile([128, COA], F32, tag="pre1")
            nc.vector.scalar_tensor_tensor(pre1[:], S, 0.125, btile[:], MULT, ADD)
            s1 = squash_scale(pre1[:], "1")
            logits = agreement(pre1[:], s1, "1")

            # iter 2
            rt2 = softmax_route(logits, "2")
            pre2 = weighted_preact(rt2, "2")
            s2 = squash_scale(pre2[:], "2")
            lc2 = agreement(pre2[:], s2, "2")
            l2 = spool.tile([128, CI * CO], F32, tag="l2")
            nc.vector.tensor_tensor(l2[:], logits[:], lc2[:], ADD)

            # iter 3
            rt3 = softmax_route(l2, "3")
            pre3 = weighted_preact(rt3, "3")
            s3 = squash_scale(pre3[:], "3")
            act = opool.tile([128, COA], F32, tag="act")
            s3b = s3[:].rearrange("p co -> p co").broadcast_to((128, CO, A))
            nc.vector.tensor_tensor(
                act[:].rearrange("p (co a) -> p co a", co=CO),
                pre3[:].rearrange("p (co a) -> p co a", co=CO),
                s3b, MULT,
            )
            nc.sync.dma_start(out_d[r], act[:])

        # main loop: blocks of rows; per block: conv all 5 images, then routing
        for r0, blen in _blocks():
            vrows = {}
            for r in range(r0, r0 + blen):
                vrows[r] = vpool.tile([128, 5 * COA], F32, tag="vrow", name=f"vrow{r}")
            for i in range(5):
                pstiles = {}
                for p in range(r0, r0 + blen):
                    pstiles[p] = pspool.tile([128, COA], F32, tag="pp", name=f"pp{i}_{p}")
                q_lo, q_hi = r0 // 4, (r0 + blen - 1) // 4 + 1
                for q in range(q_lo, q_hi + 1):
                    for kw in range(KK):
                        lhsT = qwin(i, q, kw)
                        for r in range(max(r0, 4 * q - 4), min(r0 + blen, 4 * q + 4)):
                            m = r % 4
                            sgn = 0 if r >= 4 * q else 1
                            ps = pstiles[r]
                            nc.tensor.matmul(
                                ps[:],
                                lhsT,
                                wslice(m, sgn, kw),
                                start=(sgn == 0 and kw == 0),
                                stop=(sgn == 1 and kw == KK - 1),
                                tile_position=(0, 0),
                                skip_group_check=True,
                            )
                for r in range(r0, r0 + blen):
                    nc.scalar.copy(
                        vrows[r][:, i * COA : (i + 1) * COA], pstiles[r][:]
                    )
            for r in range(r0, r0 + blen):
                routing(r, vrows[r][:])

    nc.compile()
    return nc


_NC_CACHE = None


def _get_nc():
    global _NC_CACHE
    if _NC_CACHE is None:
        _NC_CACHE = build_program()
    return _NC_CACHE


def host_prep(input_tensor, W, b):
    """Build per-core input maps."""
    x = np.asarray(input_tensor, np.float32)
    W = np.asarray(W, np.float32)
    b = np.asarray(b, np.float32)
    B, H, Wd, Ci, Ai = x.shape

    # padded input [B, H+4, W+4, Ci, Ai]
    xp = np.zeros((B, H + 4, Wd + 4, Ci, Ai), np.float32)
    xp[:, 2 : H + 2, 2 : Wd + 2] = x

    # wstk [128, 40*COA]: blocks t (partition 32t); index (m, sgn, kw)
    wstk = np.zeros((4, 32, 4, 2, KK, COA), np.float32)  # [t, a, m, sgn, kw, coa]
    for t in range(4):
        for m in range(4):
            kh = t - m
            if 0 <= kh <= 4:
                wstk[t, :, m, 0, :, :] = W[kh].transpose(1, 0, 2)  # [a, kw, coa]
            kh = 4 + t - m
            if 0 <= kh <= 4:
                wstk[t, :, m, 1, :, :] = W[kh].transpose(1, 0, 2)
    wstk = round_fp32r(wstk.reshape(128, 40 * COA))

    btile = np.broadcast_to(b.reshape(1, COA), (128, COA)).astype(np.float32).copy()

    in_maps = []
    for bb in range(B):
        for hh in range(2):
            shard = xp[bb, hh * 64 : hh * 64 + 68]  # [68, 136, Ci, Ai]
            imgs = np.concatenate(
                [
                    shard.transpose(2, 0, 3, 1),  # [Ci, 68, Ai, 136]
                    shard.sum(2).transpose(0, 2, 1)[None],
                ],
                axis=0,
            )[:, :, :, : Wd + 4]  # [5, 68, Ai, 132]
            in_maps.append(
                {
                    "xs": round_fp32r(np.ascontiguousarray(imgs)),
                    "wstk": wstk,
                    "bt": btile,
                }
            )
    return in_maps


def kernel(input_tensor, W, b):
    x = np.asarray(input_tensor, np.float32)
    B, H, Wd, Ci, Ai = x.shape
    in_maps = host_prep(x, W, b)
    nc = _get_nc()
    res = run_bass_kernel_spmd(nc, in_maps, core_ids=list(range(8)))
    out = np.zeros((B, H, Wd, CO, A), np.float32)
    k = 0
    for bb in range(B):
        for hh in range(2):
            out[bb, hh * 64 : hh * 64 + 64] = (
                res.results[k]["out"].reshape(NROWS, 128, CO, A)
            )
            k += 1
    return out



# revision 16
# speedup vs baseline: 6.3438x; 6.3438x over previous
"""ConvCapsuleLayer Trainium2 kernel: 5x5 SAME conv + 3-iter dynamic routing.

Sharding: 8 cores = batch(4) x H-halves(2). Per core: 5 images (4 input
capsules + their sum/8) of [68 rows, 32 atoms, 132 cols] (fp32r, host
pre-rounded), conv via quad-stacked K=128 matmuls. Routing is row-batched
(R=4 rows per instruction) in fp16 on DVE/Pool/ACT with fp32 accumulators;
sqrt(n2) is computed as exp(0.5*ln(n2)) so every activation function used
(Square/Ln/Exp/Copy) lives in one ACT table (no table reloads).
"""
import numpy as np
from contextlib import ExitStack

import concourse.bass as bass
import concourse.tile as tile
from concourse import bacc, mybir
from concourse.bass_utils import run_bass_kernel_spmd

KK = 5
CI, CO, A = 4, 8, 32
COA = CO * A  # 256
NROWS = 64  # output rows per core
R = 4  # rows per routing batch (quad-aligned)
F32 = mybir.dt.float32
F32R = mybir.dt.float32r
F16 = mybir.dt.float16
BF16 = mybir.dt.bfloat16
MULT = mybir.AluOpType.mult
ADD = mybir.AluOpType.add
AX = mybir.AxisListType.X
AF = mybir.ActivationFunctionType


def round_fp32r(x):
    """Round-to-nearest-even at 11 explicit mantissa bits (matches TRN2 fp32r)."""
    b = np.ascontiguousarray(x, np.float32).view(np.uint32)
    mask = np.uint32((1 << 12) - 1)
    half = np.uint32(1 << 11)
    frac = b & mask
    b2 = b & ~mask
    add = np.where(
        (frac > half) | ((frac == half) & (((b2 >> np.uint32(12)) & 1) == 1)),
        np.uint32(1 << 12),
        np.uint32(0),
    ).astype(np.uint32)
    return (b2 + add).view(np.float32)


_ACT_TABLE_KEEP = "natural_log_exp_and_others"  # contains Square, Ln, Exp, Copy


def _pin_act_tables():
    """Restrict the act-table chooser to one table covering every ACT func we
    use, so exactly one LoadActFuncSet is emitted (indices preserved)."""
    import concourse.bacc as _bacc_mod
    from concourse.hw_specs import get_activation_tables as _real

    def _filtered(arch):
        tabs = _real(arch)
        assert _ACT_TABLE_KEEP in tabs
        return {k: (v if k == _ACT_TABLE_KEEP else set()) for k, v in tabs.items()}

    _bacc_mod.get_activation_tables = _filtered


def build_program():
    _pin_act_tables()
    nc = bacc.Bacc("TRN2", target_bir_lowering=False, debug=False, num_devices=1)

    xs_d = nc.dram_tensor("xs", [5, 68, 32, 132], BF16, kind="ExternalInput").ap()
    wstk_d = nc.dram_tensor("wstk", [128, 40 * COA], BF16, kind="ExternalInput").ap()
    bt_d = nc.dram_tensor("bt", [128, COA], F16, kind="ExternalInput").ap()
    out_d = nc.dram_tensor("out", [NROWS, 128, COA], F32, kind="ExternalOutput").ap()

    with tile.TileContext(nc) as tc, ExitStack() as ctx:
        cpool = ctx.enter_context(tc.tile_pool(name="const", bufs=1))
        vpool = ctx.enter_context(tc.tile_pool(name="vblk", bufs=4))
        bigpool = ctx.enter_context(tc.tile_pool(name="big", bufs=3))
        thpool = ctx.enter_context(tc.tile_pool(name="th", bufs=3))
        hpool = ctx.enter_context(tc.tile_pool(name="half", bufs=3))
        spool = ctx.enter_context(tc.tile_pool(name="smalls", bufs=3))
        opool = ctx.enter_context(tc.tile_pool(name="outs", bufs=2))
        pspool = ctx.enter_context(tc.tile_pool(name="ps", bufs=8, space="PSUM"))

        # constants / inputs resident in SBUF (wstk first: every matmul needs it)
        wstk = cpool.tile([128, 40 * COA], BF16)
        nc.sync.dma_start(wstk[:], wstk_d[:])
        btile = cpool.tile([128, COA], F16)
        nc.sync.dma_start(btile[:], bt_d[:])
        quads = cpool.tile([128, 85 * 132], BF16)  # quad (i,q) at col (i*17+q)*132
        for i in range(5):
            src = xs_d[i].rearrange("(q r) a w -> (r a) q w", r=4)
            dst = quads[:, i * 17 * 132 : (i + 1) * 17 * 132].rearrange(
                "p (q w) -> p q w", q=17
            )
            nc.sync.dma_start(dst, src)

        def wslice(m, sgn, kw):
            o = ((m * 2 + sgn) * KK + kw) * COA
            return wstk[:, o : o + COA]

        def qwin(i, q, kw):
            o = (i * 17 + q) * 132 + kw
            return quads[:, o : o + 128]

        def bcastR(n):  # btile -> [p, n, COA] (0-stride over rows)
            return btile[:].rearrange("p (one c) -> p one c", one=1).broadcast_to(
                (128, n, COA)
            )

        def rb(t):  # [p, R*COA] tile -> [p, R, COA]
            return t[:].rearrange("p (r c) -> p r c", r=R)

        # ---- routing, split into 5 stages for software pipelining across
        # blocks (engines execute their queues in order, so cross-block
        # overlap must come from emission order) ----

        def squash_tail(n2, tag):
            # s = sqrt(n2)/(1+n2) with sqrt via exp(0.5*ln) (one ACT table)
            lnt = spool.tile([128, R * CO], F32, tag="lnt", bufs=4)
            nc.scalar.activation(lnt[:], n2[:], AF.Ln)
            rtn = spool.tile([128, R * CO], F32, tag="rtn", bufs=4)
            nc.scalar.activation(rtn[:], lnt[:], AF.Exp, scale=0.5)
            np1 = spool.tile([128, R * CO], F32, tag="np1", bufs=4)
            nc.vector.tensor_scalar_add(np1[:], n2[:], 1.0)
            dinv = spool.tile([128, R * CO], F32, tag="dinv", bufs=4)
            nc.vector.reciprocal(dinv[:], np1[:])
            s = spool.tile([128, R * CO], F16, tag="s" + tag, bufs=3)
            nc.vector.tensor_tensor(s[:], rtn[:], dinv[:], MULT)
            return s

        def sq_n2(pre):
            sq = hpool.tile([128, R * COA], F16, tag="sq", bufs=4)
            nc.scalar.activation(sq[:], pre[:], AF.Square)
            sqh = hpool.tile([128, R * COA // 2], F16, tag="sqh", bufs=4)
            sv = sq[:].rearrange("p (g a) -> p g a", a=A)
            nc.vector.tensor_tensor(
                sqh[:].rearrange("p (g a) -> p g a", a=A // 2),
                sv[:, :, 0 : A // 2], sv[:, :, A // 2 : A], ADD,
            )
            n2 = spool.tile([128, R * CO], F32, tag="n2", bufs=4)
            nc.vector.tensor_reduce(
                n2[:], sqh[:].rearrange("p (g a) -> p g a", a=A // 2), AX, ADD
            )
            return n2

        def agree_mul(Vm, pre, mul_eng):
            t = bigpool.tile([128, R * CI * COA], F16, tag="t", bufs=3)
            pv = pre[:].rearrange(
                "p (r one c) -> p r one c", r=R, one=1
            ).broadcast_to((128, R, CI, COA))
            mul_eng.tensor_tensor(
                t[:].rearrange("p (r i c) -> p r i c", r=R, i=CI),
                Vm[:].rearrange("p (r i c) -> p r i c", r=R, i=CI),
                pv, MULT,
            )
            return t

        def agree_tail(t, s, tag):
            th = thpool.tile([128, R * CI * CO * (A // 2)], F16, tag="th", bufs=3)
            tv = t[:].rearrange("p (g a) -> p g a", a=A)
            nc.vector.tensor_tensor(
                th[:].rearrange("p (g a) -> p g a", a=A // 2),
                tv[:, :, 0 : A // 2], tv[:, :, A // 2 : A], ADD,
            )
            th2 = thpool.tile([128, R * CI * CO * (A // 4)], F16, tag="th2", bufs=3)
            thv = th[:].rearrange("p (g a) -> p g a", a=A // 2)
            nc.vector.tensor_tensor(
                th2[:].rearrange("p (g a) -> p g a", a=A // 4),
                thv[:, :, 0 : A // 4], thv[:, :, A // 4 : A // 2], ADD,
            )
            lraw = spool.tile([128, R * CI * CO], F32, tag="lraw", bufs=3)
            nc.vector.tensor_reduce(
                lraw[:], th2[:].rearrange("p (g a) -> p g a", a=A // 4), AX, ADD
            )
            lc = spool.tile([128, R * CI * CO], F32, tag="lc" + tag, bufs=3)
            sv = s[:].rearrange(
                "p (r one co) -> p r one co", r=R, one=1
            ).broadcast_to((128, R, CI, CO))
            nc.vector.tensor_tensor(
                lc[:].rearrange("p (r i co) -> p r i co", r=R, i=CI),
                lraw[:].rearrange("p (r i co) -> p r i co", r=R, i=CI),
                sv, MULT,
            )
            return lc

        def softmax(l, tag):
            # rt = exp(l)/sum_co exp(l); e kept f32 (overflow safety)
            e = spool.tile([128, R * CI * CO], F32, tag="e", bufs=3)
            nc.scalar.activation(e[:], l[:], AF.Exp)
            Z = spool.tile([128, R * CI], F32, tag="Z", bufs=3)
            nc.vector.tensor_reduce(
                Z[:], e[:].rearrange("p (g co) -> p g co", co=CO), AX, ADD
            )
            zi = spool.tile([128, R * CI], F32, tag="zi", bufs=3)
            nc.vector.reciprocal(zi[:], Z[:])
            rt = spool.tile([128, R * CI * CO], F16, tag="rt" + tag, bufs=3)
            zv = zi[:].rearrange(
                "p (g one) -> p g one", one=1
            ).broadcast_to((128, R * CI, CO))
            nc.vector.tensor_tensor(
                rt[:].rearrange("p (g co) -> p g co", co=CO),
                e[:].rearrange("p (g co) -> p g co", co=CO), zv, MULT,
            )
            return rt

        def weighted(Vm, rt, tag):
            # pre[r,co,a] = sum_ci rt[r,ci,co]*Vm[r,ci,co,a] + b
            rV = bigpool.tile([128, R * CI * COA], F16, tag="rv", bufs=3)
            rv_b = rt[:].rearrange(
                "p (g one) -> p g one", one=1
            ).broadcast_to((128, R * CI * CO, A))
            nc.gpsimd.tensor_tensor(
                rV[:].rearrange("p (g a) -> p g a", a=A),
                Vm[:].rearrange("p (g a) -> p g a", a=A), rv_b, MULT,
            )
            rvv = rV[:].rearrange("p (r i c) -> p r i c", r=R, i=CI)
            h1 = hpool.tile([128, R * COA], F16, tag="h1", bufs=3)
            nc.vector.tensor_tensor(rb(h1), rvv[:, :, 0], rvv[:, :, 1], ADD)
            h2 = hpool.tile([128, R * COA], F16, tag="h2", bufs=3)
            nc.vector.tensor_tensor(rb(h2), rvv[:, :, 2], rvv[:, :, 3], ADD)
            h12 = hpool.tile([128, R * COA], F16, tag="h12", bufs=3)
            nc.vector.tensor_tensor(h12[:], h1[:], h2[:], ADD)
            pre = hpool.tile([128, R * COA], F16, tag="pre" + tag, bufs=3)
            nc.vector.tensor_tensor(rb(pre), rb(h12), bcastR(R), ADD)
            return pre

        def stage1(st):
            # iter 1: uniform route -> pre1 = S/8 + b (1/8 folded on host)
            pre1 = hpool.tile([128, R * COA], F16, tag="pre1", bufs=3)
            nc.vector.tensor_tensor(rb(pre1), rb(st["Vs"]), bcastR(R), ADD)
            n2 = sq_n2(pre1)
            t1 = agree_mul(st["Vm"], pre1, nc.vector)
            s1 = squash_tail(n2, "1")
            st["lgt"] = agree_tail(t1, s1, "1")

        def stage2(st):
            rt2 = softmax(st["lgt"], "2")
            st["pre2"] = weighted(st["Vm"], rt2, "2")

        def stage3(st):
            pre2 = st.pop("pre2")
            n2 = sq_n2(pre2)
            t2 = agree_mul(st["Vm"], pre2, nc.gpsimd)
            s2 = squash_tail(n2, "2")
            lc2 = agree_tail(t2, s2, "2")
            l3 = spool.tile([128, R * CI * CO], F32, tag="l3", bufs=3)
            nc.vector.tensor_tensor(l3[:], st.pop("lgt")[:], lc2[:], ADD)
            st["l3"] = l3

        def stage4(st):
            rt3 = softmax(st.pop("l3"), "3")
            st["pre3"] = weighted(st["Vm"], rt3, "3")

        def stage5(st):
            pre3 = st.pop("pre3")
            n2 = sq_n2(pre3)
            s3 = squash_tail(n2, "3")
            act = opool.tile([128, R * COA], F32, tag="act", bufs=2)
            s3v = s3[:].rearrange(
                "p (g one) -> p g one", one=1
            ).broadcast_to((128, R * CO, A))
            nc.vector.tensor_tensor(
                act[:].rearrange("p (g a) -> p g a", a=A),
                pre3[:].rearrange("p (g a) -> p g a", a=A), s3v, MULT,
            )
            for j in range(R):
                nc.sync.dma_start(
                    out_d[st["r0"] + j], act[:, j * COA : (j + 1) * COA]
                )

        STAGES = [stage1, stage2, stage3, stage4, stage5]

        # main loop: quad-aligned blocks of R rows; emission software-pipelined
        # (conv of block k+1 is emitted before routing of block k)
        def conv_block(r0):
            Vm = vpool.tile([128, R * CI * COA], F16, tag="vm", name=f"vm{r0}")
            Vs = vpool.tile([128, R * COA], F16, tag="vs", name=f"vs{r0}")
            for i in range(5):
                pstiles = {}
                for p in range(r0, r0 + R):
                    pstiles[p] = pspool.tile([128, COA], F32, tag="pp", name=f"pp{i}_{p}")
                q_lo, q_hi = r0 // 4, (r0 + R - 1) // 4 + 1
                for q in range(q_lo, q_hi + 1):
                    for kw in range(KK):
                        lhsT = qwin(i, q, kw)
                        for r in range(max(r0, 4 * q - 4), min(r0 + R, 4 * q + 4)):
                            m = r % 4
                            sgn = 0 if r >= 4 * q else 1
                            ps = pstiles[r]
                            nc.tensor.matmul(
                                ps[:],
                                lhsT,
                                wslice(m, sgn, kw),
                                start=(sgn == 0 and kw == 0),
                                stop=(sgn == 1 and kw == KK - 1),
                                tile_position=(0, 0),
                                skip_group_check=True,
                            )
                for r in range(r0, r0 + R):
                    if i < CI:
                        dst = Vm[:, ((r - r0) * CI + i) * COA : ((r - r0) * CI + i + 1) * COA]
                    else:
                        dst = Vs[:, (r - r0) * COA : (r - r0 + 1) * COA]
                    nc.scalar.copy(dst, pstiles[r][:])
            return Vm, Vs

        # wavefront emission: stage j of block k at round r = 2k + j; deeper
        # stages of older blocks are emitted first within a round, so ~3
        # blocks' routing interleaves in every engine's (in-order) queue.
        blocks = list(range(0, NROWS, R))
        NB = len(blocks)
        states = {}
        OFF = 2  # stage offset between consecutive blocks
        n_rounds = OFF * (NB - 1) + len(STAGES)
        for r in range(n_rounds):
            # conv for block k lands one block-offset ahead of its stage1
            for k in range(NB):
                if max(0, OFF * k - OFF) == r and k not in states:
                    Vm, Vs = conv_block(blocks[k])
                    states[k] = {"r0": blocks[k], "Vm": Vm, "Vs": Vs}
            # older blocks' deeper stages first (k ascending -> j descending)
            for k in range(NB):
                j = r - OFF * k
                if 0 <= j < len(STAGES):
                    STAGES[j](states[k])

    nc.compile()
    return nc


_NC_CACHE = None


def _get_nc():
    global _NC_CACHE
    if _NC_CACHE is None:
        _NC_CACHE = build_program()
    return _NC_CACHE


def host_prep(input_tensor, W, b):
    """Build per-core input maps."""
    x = np.asarray(input_tensor, np.float32)
    W = np.asarray(W, np.float32)
    b = np.asarray(b, np.float32)
    B, H, Wd, Ci, Ai = x.shape

    # padded input [B, H+4, W+4, Ci, Ai]
    xp = np.zeros((B, H + 4, Wd + 4, Ci, Ai), np.float32)
    xp[:, 2 : H + 2, 2 : Wd + 2] = x

    # wstk [128, 40*COA]: blocks t (partition 32t); index (m, sgn, kw)
    wstk = np.zeros((4, 32, 4, 2, KK, COA), np.float32)  # [t, a, m, sgn, kw, coa]
    for t in range(4):
        for m in range(4):
            kh = t - m
            if 0 <= kh <= 4:
                wstk[t, :, m, 0, :, :] = W[kh].transpose(1, 0, 2)  # [a, kw, coa]
            kh = 4 + t - m
            if 0 <= kh <= 4:
                wstk[t, :, m, 1, :, :] = W[kh].transpose(1, 0, 2)
    import ml_dtypes
    wstk = wstk.reshape(128, 40 * COA).astype(ml_dtypes.bfloat16)

    btile = np.broadcast_to(b.reshape(1, COA), (128, COA)).astype(np.float16).copy()

    in_maps = []
    for bb in range(B):
        for hh in range(2):
            shard = xp[bb, hh * 64 : hh * 64 + 68]  # [68, 136, Ci, Ai]
            imgs = np.concatenate(
                [
                    shard.transpose(2, 0, 3, 1),  # [Ci, 68, Ai, 136]
                    (shard.sum(2) * 0.125).transpose(0, 2, 1)[None],
                ],
                axis=0,
            )[:, :, :, : Wd + 4]  # [5, 68, Ai, 132]
            in_maps.append(
                {
                    "xs": np.ascontiguousarray(imgs).astype(ml_dtypes.bfloat16),
                    "wstk": wstk,
                    "bt": btile,
                }
            )
    return in_maps


def kernel(input_tensor, W, b):
    x = np.asarray(input_tensor, np.float32)
    B, H, Wd, Ci, Ai = x.shape
    in_maps = host_prep(x, W, b)
    nc = _get_nc()
    res = run_bass_kernel_spmd(nc, in_maps, core_ids=list(range(8)))
    out = np.zeros((B, H, Wd, CO, A), np.float32)
    k = 0
    for bb in range(B):
        for hh in range(2):
            out[bb, hh * 64 : hh * 64 + 64] = (
                res.results[k]["out"].reshape(NROWS, 128, CO, A)
            )
            k += 1
    return out
